# revision 11
# baseline (speedup 1.0000x reference)
"""Trainium2 Bass kernel: transformer block (QKV proj + MHA + residual + LN +
MLP(relu) residual + LN) for B=2, S=4096, D=512, H=8.

Sharding: data-parallel over (batch, query-row-block) — 8 cores x 1024 query
rows. Each core recomputes K/V projections for its batch (4 cores share a
batch), attends over all 4096 keys, and runs the per-row tail. No cross-core
communication.

Layouts: feature-major ("T" = [d, rows]) so projection/attention matmuls chain
without re-transposing. Softmax sums come free from a ones-column appended to
each V tile. fp32r matmuls (full PE rate at moving-dim >= 256).
"""

import math

import numpy as np

import concourse.bass as bass
import concourse.mybir as mybir
import concourse.tile as tile
from concourse.masks import make_identity

# ---------------------------------------------------------------------------
# Workaround: this walrus build rejects >1 sync-wait on the TileContext exit
# drain (CoreV3 setupSyncWait "Too many sync wait commands"). Split the waits
# across single-wait NOPs.
_orig_drain_and_barrier = tile.TileContext._drain_and_barrier


def _split_drain_and_barrier(self, tick_clock, wait_clock):
    from concourse.tile import ScopedClock

    nc = self.nc
    drain_inst = nc.sync.drain()
    wait_clock.add_sem_waits(
        drain_inst.ins, ScopedClock({None: tick_clock.global_clock})
    )
    si = drain_inst.ins.sync_info
    waits = list(si.on_wait) if si and si.on_wait else []
    if len(waits) > 1:
        si.on_wait = waits[:1]
        for w in waits[1:]:
            nop = nc.sync.nop(nofuse=True, hint="drain_wait_split")
            nop.ins.sync_info = mybir.SyncInfo(on_wait=[w], on_update=[])
    nc.all_engine_barrier()
    assert self.sems is not None
    popped = nc._tile_sem_poison_stack.pop()
    assert popped is self._sem_poison
    nc.clear_and_free_semaphores(list(self.sems.allocated().values()))
    nc.all_engine_barrier()


tile.TileContext._drain_and_barrier = _split_drain_and_barrier


def _split_multi_waits(nc, limit=1):
    """walrus CoreV3 codegen caps sync-waits per instruction descriptor; hoist
    excess waits onto fresh NOPs inserted just before the instruction on the
    same engine queue."""
    ctr = [0]

    def mknop(engine, wait):
        ctr[0] += 1
        nop = mybir.InstNoOp(name=f"WSPLIT-{ctr[0]}", ins=[], outs=[])
        nop.engine = engine
        nop.sync_info = mybir.SyncInfo(on_wait=[wait], on_update=[])
        return nop

    nsplit = 0
    for f in nc.m.functions:
        for bb in f.blocks:
            insts = bb.instructions
            i = 0
            while i < len(insts):
                ins = insts[i]
                si = getattr(ins, "sync_info", None)
                if si is not None and si.on_wait and len(si.on_wait) > limit:
                    waits = list(si.on_wait)
                    si.on_wait = waits[-limit:]
                    pre = [mknop(ins.engine, w) for w in waits[:-limit]]
                    for j, p in enumerate(pre):
                        insts.insert(i + j, p)
                        nc.register_instruction(p, overwrite=True)
                    i += len(pre)
                    nsplit += 1
                i += 1
    return nsplit, ctr[0]
# ---------------------------------------------------------------------------

B, S, D, H, DH = 2, 4096, 512, 8, 64
P = 128
NC = 8          # cores
RPC = 1024      # query rows per core
NKC = D // P    # 4 contraction chunks of 128
EPS = 1e-5
SCALE = 1.0 / math.sqrt(D)

F32 = mybir.dt.float32
F32R = mybir.dt.float32r
BF16 = mybir.dt.bfloat16
ALU = mybir.AluOpType
AF = mybir.ActivationFunctionType


def r(ap):
    """view an fp32 AP as float32r for full-rate PE matmuls"""
    return ap.bitcast(F32R)


def build_nc():
    nc = bass.Bass()

    Qr = nc.dram_tensor("Qr", [RPC, D], F32, kind="ExternalInput")
    Kb = nc.dram_tensor("Kb", [S, D], F32, kind="ExternalInput")
    Wq = nc.dram_tensor("Wq", [D, D], F32R, kind="ExternalInput")
    Wk = nc.dram_tensor("Wk", [D, D], F32R, kind="ExternalInput")
    Wv = nc.dram_tensor("Wv", [D, D], F32R, kind="ExternalInput")
    Wo = nc.dram_tensor("Wo", [D, D], F32R, kind="ExternalInput")
    bq = nc.dram_tensor("bq", [D], F32, kind="ExternalInput")
    bk = nc.dram_tensor("bk", [D], F32, kind="ExternalInput")
    bv = nc.dram_tensor("bv", [D], F32, kind="ExternalInput")
    bo = nc.dram_tensor("bo", [D], F32, kind="ExternalInput")
    g0 = nc.dram_tensor("g0", [D], F32, kind="ExternalInput")
    b0 = nc.dram_tensor("b0", [D], F32, kind="ExternalInput")
    g1 = nc.dram_tensor("g1", [D], F32, kind="ExternalInput")
    b1 = nc.dram_tensor("b1", [D], F32, kind="ExternalInput")
    Oo = nc.dram_tensor("O", [RPC, D], F32, kind="ExternalOutput")

    def bcast_ap(dram_vec):
        # [cols] dram vector -> [P, cols] partition-broadcast AP
        a = dram_vec[:]
        return bass.AP(
            tensor=a.tensor,
            offset=a.offset,
            ap=[[0, P]] + list(a.ap),
        )

    def chunked_ap(dram_vec):
        # [D] dram vector -> [P, NKC]: partition = idx within 128-chunk
        return dram_vec.rearrange("(c p) -> p c", p=P)

    with tile.TileContext(nc) as tc:
        with (
            tc.tile_pool(name="consts", bufs=1) as consts,
            tc.tile_pool(name="kT_p", bufs=1) as kT_p,
            tc.tile_pool(name="vx_p", bufs=1) as vx_p,
            tc.tile_pool(name="qT_p", bufs=1) as qT_p,
            tc.tile_pool(name="w_p", bufs=2) as w_p,
            tc.tile_pool(name="ktc_p", bufs=2) as ktc_p,
            tc.tile_pool(name="kload_p", bufs=2) as kload_p,
            tc.tile_pool(name="at_p", bufs=3) as at_p,
            tc.tile_pool(name="ot_p", bufs=2) as ot_p,
            tc.tile_pool(name="opre_p", bufs=4) as opre_p,
            tc.tile_pool(name="tail_p", bufs=2) as tail_p,
            tc.tile_pool(name="stat_p", bufs=4) as stat_p,
            tc.tile_pool(name="ps_t", bufs=2, space="PSUM") as ps_t,
            tc.tile_pool(name="ps_mm", bufs=2, space="PSUM") as ps_mm,
            tc.tile_pool(name="ps_sc", bufs=2, space="PSUM") as ps_sc,
            tc.tile_pool(name="ps_o", bufs=2, space="PSUM") as ps_o,
        ):
            # ---- constants ----
            ident = consts.tile([P, P], F32)
            make_identity(nc, ident)
            eps_t = consts.tile([P, 1], F32)
            nc.vector.memset(eps_t, EPS)

            wk_t = w_p.tile([P, NKC, D], F32R, tag="w", name="wk_t")
            nc.sync.dma_start(wk_t, Wk.rearrange("(c p) n -> p c n", p=P))
            wv_t = w_p.tile([P, NKC, D], F32R, tag="w", name="wv_t")
            nc.sync.dma_start(wv_t, Wv.rearrange("(c p) n -> p c n", p=P))
            # per-partition biases for feature-major adds: [P, NKC]
            bias_sb = consts.tile([P, 3, NKC], F32)
            for wi, bvec in enumerate((bq, bk, bv)):
                nc.gpsimd.dma_start(bias_sb[:, wi], chunked_ap(bvec))
            # free-dim broadcast rows: [P, D]
            bvb = consts.tile([P, D], F32)
            bob = consts.tile([P, D], F32)
            g0b = consts.tile([P, D], F32)
            b0b = consts.tile([P, D], F32)
            g1b = consts.tile([P, D], F32)
            b1b = consts.tile([P, D], F32)
            for t, v in ((bvb, bv), (bob, bo), (g0b, g0), (b0b, b0),
                         (g1b, g1), (b1b, b1)):
                nc.gpsimd.dma_start(t, bcast_ap(v))

            # ---- persistent activations ----
            kT = kT_p.tile([P, NKC, S], F32R)          # (K Wk + bk)^T
            v_ext = vx_p.tile([P, S // P, H, DH + 1], BF16)  # V rows + ones col
            qT = qT_p.tile([P, NKC, RPC], F32R)        # (Q Wq + bq)^T
            nc.vector.memset(v_ext[:, :, :, DH:DH + 1], 1.0)

            # ---- phase A: K -> K^T -> kT, v_ext (streamed, 256 rows/chunk) ----
            SC2 = 256
            for sc2 in range(S // SC2):
                ktiles = []
                for half in range(2):
                    kt = kload_p.tile([P, D], F32, tag="kload")
                    nc.sync.dma_start(
                        kt, Kb[sc2 * SC2 + half * P: sc2 * SC2 + (half + 1) * P, :]
                    )
                    ktiles.append(kt)
                KTc = ktc_p.tile([P, NKC, SC2], F32R, tag="ktc")
                for half in range(2):
                    for kc in range(NKC):
                        pt = ps_t.tile([P, P], F32, tag="pt")
                        nc.tensor.transpose(
                            pt, ktiles[half][:, kc * P:(kc + 1) * P], ident
                        )
                        nc.vector.tensor_copy(
                            KTc[:, kc, half * P:(half + 1) * P], pt
                        )
                # kT columns for this s-range (+bk)
                for ci in range(NKC):
                    pp = ps_mm.tile([P, SC2], F32, tag="pmm")
                    for kc in range(NKC):
                        nc.tensor.matmul(
                            pp,
                            lhsT=wk_t[:, kc, ci * P:(ci + 1) * P],
                            rhs=KTc[:, kc],
                            start=(kc == 0), stop=(kc == NKC - 1),
                        )
                    nc.vector.tensor_scalar_add(
                        kT[:, ci, sc2 * SC2:(sc2 + 1) * SC2], pp,
                        bias_sb[:, 1, ci:ci + 1],
                    )
                # v rows for this s-range (+bv), per 128-row block
                for half in range(2):
                    pv = ps_mm.tile([P, D], F32, tag="pmm")
                    for kc in range(NKC):
                        nc.tensor.matmul(
                            pv,
                            lhsT=KTc[:, kc, half * P:(half + 1) * P],
                            rhs=wv_t[:, kc],
                            start=(kc == 0), stop=(kc == NKC - 1),
                        )
                    sidx = sc2 * 2 + half
                    for h in range(H):
                        nc.vector.scalar_tensor_tensor(
                            out=v_ext[:, sidx, h, 0:DH],
                            in0=pv[:, h * DH:(h + 1) * DH],
                            scalar=1.0,
                            in1=bvb[:, h * DH:(h + 1) * DH],
                            op0=ALU.mult, op1=ALU.add,
                        )

            # ---- phase B: Q -> Q^T -> qT (+bq) ----
            wq_t = w_p.tile([P, NKC, D], F32R, tag="w", name="wq_t")
            nc.sync.dma_start(wq_t, Wq.rearrange("(c p) n -> p c n", p=P))
            for rc2 in range(RPC // SC2):
                qtiles = []
                for half in range(2):
                    qt = kload_p.tile([P, D], F32, tag="kload")
                    nc.sync.dma_start(
                        qt, Qr[rc2 * SC2 + half * P: rc2 * SC2 + (half + 1) * P, :]
                    )
                    qtiles.append(qt)
                QTc = ktc_p.tile([P, NKC, SC2], F32R, tag="ktc")
                for half in range(2):
                    for kc in range(NKC):
                        pt = ps_t.tile([P, P], F32, tag="pt")
                        nc.tensor.transpose(
                            pt, qtiles[half][:, kc * P:(kc + 1) * P], ident
                        )
                        nc.vector.tensor_copy(
                            QTc[:, kc, half * P:(half + 1) * P], pt
                        )
                for ci in range(NKC):
                    pp = ps_mm.tile([P, SC2], F32, tag="pmm")
                    for kc in range(NKC):
                        nc.tensor.matmul(
                            pp,
                            lhsT=wq_t[:, kc, ci * P:(ci + 1) * P],
                            rhs=QTc[:, kc],
                            start=(kc == 0), stop=(kc == NKC - 1),
                        )
                    nc.vector.tensor_scalar_add(
                        qT[:, ci, rc2 * SC2:(rc2 + 1) * SC2], pp,
                        bias_sb[:, 0, ci:ci + 1],
                    )

            # ---- phase C: attention + tail, per 512-row block ----
            wo_t = w_p.tile([P, NKC, D], F32R, tag="w", name="wo_t")
            nc.sync.dma_start(wo_t, Wo.rearrange("(c p) n -> p c n", p=P))
            RB = 512
            for rb in range(RPC // RB):
                opre = [opre_p.tile([P, H, DH], F32, tag="opre", name=f"opre_{rb}_{i}")
                        for i in range(RB // P)]
                for h in range(H):
                    ci, po = h // 2, (h % 2) * DH
                    oacc = ps_o.tile([DH + 1, RB], F32, tag="po")
                    for sc in range(S // P):
                        ps = ps_sc.tile([P, RB], F32, tag="psc")
                        nc.tensor.matmul(
                            ps,
                            lhsT=kT[po:po + DH, ci, sc * P:(sc + 1) * P],
                            rhs=qT[po:po + DH, ci, rb * RB:(rb + 1) * RB],
                            start=True, stop=True,
                        )
                        at = at_p.tile([P, RB], BF16, tag="at")
                        nc.scalar.activation(at, ps, AF.Exp, scale=SCALE)
                        nc.tensor.matmul(
                            oacc,
                            lhsT=v_ext[:, sc, h],
                            rhs=at,
                            start=(sc == 0), stop=(sc == S // P - 1),
                        )
                    # head postprocess: transpose + normalize into opre
                    ot = ot_p.tile([DH + 1, RB], F32, tag="ot")
                    nc.vector.tensor_copy(ot, oacc)
                    for rc in range(RB // P):
                        pt = ps_t.tile([P, P], F32, tag="pt")
                        nc.tensor.transpose(
                            pt[:, 0:DH + 1], ot[:, rc * P:(rc + 1) * P],
                            ident[0:DH + 1, 0:DH + 1]
                        )
                        rec = stat_p.tile([P, 1], F32, tag="rec")
                        nc.vector.reciprocal(rec, pt[:, DH:DH + 1])
                        nc.vector.tensor_scalar_mul(
                            opre[rc][:, h], pt[:, 0:DH], rec
                        )

                # tail for this row block, per 128-row chunk
                for rc in range(RB // P):
                    gr = rb * (RB // P) + rc  # global 128-row chunk id
                    x = opre[rc].rearrange("p h d -> p (h d)")
                    # + q residual (transpose qT chunks)
                    for kc in range(NKC):
                        pt = ps_t.tile([P, P], F32, tag="pt")
                        nc.tensor.transpose(
                            pt, qT.bitcast(F32)[:, kc, gr * P:(gr + 1) * P], ident
                        )
                        nc.vector.tensor_add(
                            x[:, kc * P:(kc + 1) * P],
                            x[:, kc * P:(kc + 1) * P], pt
                        )

                    def layernorm(dst, src, gb, bb):
                        st6 = stat_p.tile([P, 6], F32, tag="st6")
                        nc.vector.bn_stats(st6, src)
                        mv = stat_p.tile([P, 2], F32, tag="mv")
                        nc.vector.bn_aggr(mv, st6)
                        negmean = stat_p.tile([P, 1], F32, tag="negmean")
                        nc.scalar.mul(negmean, mv[:, 0:1], -1.0)
                        srt = stat_p.tile([P, 1], F32, tag="srt")
                        nc.scalar.activation(srt, mv[:, 1:2], AF.Sqrt, bias=eps_t)
                        rstd = stat_p.tile([P, 1], F32, tag="rstd")
                        nc.vector.reciprocal(rstd, srt)
                        xc = tail_p.tile([P, D], F32, tag="xc")
                        nc.scalar.activation(xc, src, AF.Identity, bias=negmean)
                        nc.vector.scalar_tensor_tensor(
                            out=dst, in0=xc, scalar=rstd, in1=gb,
                            op0=ALU.mult, op1=ALU.mult,
                        )
                        nc.vector.tensor_add(dst, dst, bb)

                    ln0 = tail_p.tile([P, D], F32, tag="ln0")
                    layernorm(ln0, x, g0b, b0b)
                    # mlp: ln0 @ Wo + bo, relu, + ln0
                    lnT = tail_p.tile([P, NKC, P], F32R, tag="lnT")
                    for kc in range(NKC):
                        pt = ps_t.tile([P, P], F32, tag="pt")
                        nc.tensor.transpose(
                            pt, ln0[:, kc * P:(kc + 1) * P], ident
                        )
                        nc.vector.tensor_copy(lnT[:, kc], pt)
                    pm = ps_mm.tile([P, D], F32, tag="pmm")
                    for kc in range(NKC):
                        nc.tensor.matmul(
                            pm, lhsT=lnT[:, kc], rhs=wo_t[:, kc],
                            start=(kc == 0), stop=(kc == NKC - 1),
                        )
                    mlp = tail_p.tile([P, D], F32, tag="xc", name="mlp_t")
                    nc.vector.scalar_tensor_tensor(
                        out=mlp, in0=pm, scalar=1.0, in1=bob,
                        op0=ALU.mult, op1=ALU.add,
                    )
                    nc.vector.tensor_scalar_max(mlp, mlp, 0.0)
                    nc.vector.tensor_add(mlp, mlp, ln0)
                    out_t = tail_p.tile([P, D], F32, tag="xc", name="out_t")
                    layernorm(out_t, mlp, g1b, b1b)
                    nc.sync.dma_start(Oo[gr * P:(gr + 1) * P, :], out_t)

    nsplit, nnops = _split_multi_waits(nc)
    print(f"wait-split: {nsplit} instructions, {nnops} nops inserted")
    return nc


_cached = {}


def _get_nc():
    if "nc" not in _cached:
        _cached["nc"] = build_nc()
    return _cached["nc"]


def kernel(Q, K, Wq, bq, Wk, bk, Wv, bv, Wo, bo, g0, b0, g1, b1):
    from concourse.bass_utils import run_bass_kernel_spmd

    nc = _get_nc()
    Q = np.ascontiguousarray(Q, dtype=np.float32)
    K = np.ascontiguousarray(K, dtype=np.float32)
    shared = {
        "Wq": np.ascontiguousarray(Wq, np.float32),
        "Wk": np.ascontiguousarray(Wk, np.float32),
        "Wv": np.ascontiguousarray(Wv, np.float32),
        "Wo": np.ascontiguousarray(Wo, np.float32),
        "bq": np.ascontiguousarray(bq, np.float32),
        "bk": np.ascontiguousarray(bk, np.float32),
        "bv": np.ascontiguousarray(bv, np.float32),
        "bo": np.ascontiguousarray(bo, np.float32),
        "g0": np.ascontiguousarray(g0, np.float32),
        "b0": np.ascontiguousarray(b0, np.float32),
        "g1": np.ascontiguousarray(g1, np.float32),
        "b1": np.ascontiguousarray(b1, np.float32),
    }
    in_maps = []
    for c in range(NC):
        b, roff = c // 4, (c % 4) * RPC
        in_maps.append(
            dict(shared,
                 Qr=np.ascontiguousarray(Q[b, roff:roff + RPC]),
                 Kb=np.ascontiguousarray(K[b]))
        )
    res = run_bass_kernel_spmd(nc, in_maps, core_ids=list(range(NC)))
    out = np.empty((B, S, D), np.float32)
    for c in range(NC):
        b, roff = c // 4, (c % 4) * RPC
        out[b, roff:roff + RPC] = res.results[c]["O"]
    return out
